# revision 1
# baseline (speedup 1.0000x reference)
"""Bass/Tile kernel for nn_ModelGCNConcAfterFrame: 8-core data-parallel over videos.

Per core (NV=2 videos): stage-1 object graphs (64 graphs x 50 nodes, padded to
64-slots, 2 graphs per 128-partition chunk), stage-2 frame graphs (2 x 32),
classifier. See design notes in comments.
"""
import sys
sys.path.insert(0, '/opt/trn_rl_repo')
from contextlib import ExitStack
import numpy as np
import concourse.bass as bass
import concourse.bacc as bacc
import concourse.tile as tile
from concourse import mybir
from concourse.bass_utils import run_bass_kernel_spmd

F32 = mybir.dt.float32
F32R = mybir.dt.float32r
BF16 = mybir.dt.bfloat16
AF = mybir.ActivationFunctionType
ALU = mybir.AluOpType

N_CORES = 8
NV = 2
FR, B, NF, L, NC = 32, 50, 768, 2, 400
G1 = NV * FR          # 64 object graphs / core
NCH = G1 // 2         # 32 chunks (2 graphs of 64-padded nodes each)
KC = NF // 128        # 6
WIN = 400             # prologue window = 8 graphs
NWIN = G1 * B // WIN  # 8
GW = WIN // B         # graphs per window = 8
LN_EPS = 1e-5
L1_EPS = 1e-12
NSPLITS = ((0, 512), (512, 256))  # AXW free-dim splits (bank-aligned)
SQW = 128  # sqT width (adj blockdiag tile columns)


def build(ln_trivial=True, dt_mm=F32R, dt_ax=F32, repeats=1):
    """dt_mm: feature-contraction matmuls (TMP/dotT/AXW + stage2);
    dt_ax: node-contraction matmuls (AX, rs, mean) and activations storage.
    repeats: emit the whole kernel R times (steady-state timing harness)."""
    nc = bacc.Bacc("TRN2", target_bir_lowering=False, debug=False,
                   num_devices=N_CORES)

    def din(name, shape, dt):
        return nc.dram_tensor(name, shape, dt, kind="ExternalInput").ap()

    def dout(name, shape, dt):
        return nc.dram_tensor(name, shape, dt, kind="ExternalOutput").ap()

    mmc = (lambda ap: ap.bitcast(F32)) if dt_mm == F32R else (lambda ap: ap)
    axc = (lambda ap: ap.bitcast(F32)) if dt_ax == F32R else (lambda ap: ap)
    dt_sm = BF16 if dt_ax == BF16 else F32
    x0f_d = din("x0f", [KC, 128, G1 * B], dt_mm)
    x0r_d = din("x0r", [NCH, 128, NF], dt_ax)
    mM_d = din("mM", [KC, 128, NF], dt_mm)
    gw_d = din("gw", [L, KC, 128, NF], dt_mm)
    f1t_d = din("f1t", [2 * KC, 128, NF], F32)
    f1b_d = din("f1b", [KC, 128, 1], F32)
    f2t_d = din("f2t", [KC, 128, NC], F32)
    f2b_d = din("f2b", [4, 128, 1], F32)
    fg_d = din("fg", [KC, 128, NV], F32)
    ones_d = din("ones", [128, 1], dt_sm)
    mv_d = din("mv", [128, NV], dt_sm)
    mv2_d = din("mv2", [128, NV], dt_sm)
    ident_d = din("ident", [128, 128], F32)
    zeros_d = din("zeros", [128, NCH * SQW], dt_ax)
    if not ln_trivial:
        lng_d = din("lng", [L, 128, NF], F32)
        lnb_d = din("lnb", [L, 128, NF], F32)

    lg_out = dout("lgT", [4, 128, NV], F32)
    x3_out = dout("x3T", [KC, 128, NV], F32)

    def emit(tc, ctx, R):
        P = lambda n: f"{n}_{R}"
        const = ctx.enter_context(tc.tile_pool(name=P("const"), bufs=1))
        wpool = ctx.enter_context(tc.tile_pool(name=P("wpool"), bufs=1))
        sqp = ctx.enter_context(tc.tile_pool(name=P("sqp"), bufs=1))

        ones = const.tile([128, 1], dt_sm)
        nc.sync.dma_start(ones[:], ones_d[:])
        mv = const.tile([128, NV], dt_sm)
        nc.sync.dma_start(mv[:], mv_d[:])
        mv2 = const.tile([128, NV], dt_sm)
        nc.sync.dma_start(mv2[:], mv2_d[:])
        ident = const.tile([128, 128], F32)
        nc.sync.dma_start(ident[:], ident_d[:])
        epsb = const.tile([128, 1], F32)
        nc.gpsimd.memset(epsb[:], LN_EPS)

        mMt = wpool.tile([128, KC, NF], dt_mm)
        nc.sync.dma_start(mMt[:], mM_d.rearrange("k p n -> p k n"))
        gwt = wpool.tile([128, L, KC, NF], dt_mm)
        nc.sync.dma_start(gwt[:], gw_d.rearrange("l k p n -> p l k n"))
        if not ln_trivial:
            lngt = wpool.tile([128, L, NF], F32)
            nc.sync.dma_start(lngt[:], lng_d.rearrange("l p n -> p l n"))
            lnbt = wpool.tile([128, L, NF], F32)
            nc.sync.dma_start(lnbt[:], lnb_d.rearrange("l p n -> p l n"))

        sqT = sqp.tile([128, NCH, SQW], dt_ax)
        invrs = sqp.tile([128, NCH], F32)
        sqT2 = sqp.tile([128, SQW], dt_ax)
        if dt_ax != F32R:
            nc.gpsimd.memset(sqT[:], 0.0)
            nc.gpsimd.memset(sqT2[:], 0.0)
        else:
            nc.sync.dma_start(sqT[:], zeros_d[:])
            nc.sync.dma_start(sqT2[:], zeros_d[:, 0:SQW])
        invrs2 = sqp.tile([128, 1], F32)

        # ================= PHASE A =================
        with tc.tile_pool(name=P("x0fw"), bufs=3) as x0fp, \
             tc.tile_pool(name=P("tmpfw"), bufs=3) as tmpfp, \
             tc.tile_pool(name=P("pA"), bufs=3, space="PSUM") as pA, \
             tc.tile_pool(name=P("pDot"), bufs=2, space="PSUM") as pDot, \
             tc.tile_pool(name=P("tA"), bufs=4) as tA:
            for w in range(NWIN):
                x0fw = x0fp.tile([128, KC, WIN], dt_mm, tag="x0fw")
                for k in range(KC):
                    nc.sync.dma_start(x0fw[:, k, :],
                                      x0f_d[k, :, w * WIN:(w + 1) * WIN])
                tmpfw = tmpfp.tile([128, KC, WIN], dt_mm, tag="tmpfw")
                for lch in range(KC):
                    ps = pA.tile([128, 512], F32, tag="pA")
                    for k in range(KC):
                        nc.tensor.matmul(ps[:, :WIN],
                                         mMt[:, k, lch * 128:(lch + 1) * 128],
                                         x0fw[:, k, :],
                                         start=(k == 0), stop=(k == KC - 1))
                    nc.vector.tensor_copy(tmpfw[:, lch, :], ps[:, :WIN])
                for gi in range(GW):
                    g = w * GW + gi
                    c, odd = divmod(g, 2)
                    lo = gi * B
                    po = 64 * odd
                    pd = pDot.tile([128, B], F32, tag="pDot")
                    for k in range(KC):
                        nc.tensor.matmul(pd[po:po + B, :],
                                         mmc(x0fw[:, k, lo:lo + B]),
                                         mmc(tmpfw[:, k, lo:lo + B]),
                                         start=(k == 0), stop=(k == KC - 1),
                                         tile_position=(0, po))
                    nc.scalar.activation(sqT[po:po + B, c, po:po + B],
                                         pd[po:po + B, :], AF.Square)
                    if odd:
                        pr = pDot.tile([128, 1], F32, tag="pDot")
                        nc.tensor.matmul(pr[:], axc(sqT[:, c, 0:128]),
                                         ones[:], start=True, stop=True)
                        t = tA.tile([128, 1], F32, tag="rsmax")
                        nc.vector.tensor_scalar_max(t[:], pr[:], L1_EPS)
                        nc.vector.reciprocal(invrs[:, c:c + 1], t[:])

        # ================= PHASE B =================
        def gcn_layer(i, xr, c, sqT_ap, invrs_ap, axfp, pAX, pH, hcp, lnp, xnp):
            """One GCN layer on one 128-row chunk. Returns new x tile (dt_ax)."""
            pax = pAX.tile([128, KC, SQW], F32, tag="pax")
            for ph in range(KC):
                nc.tensor.matmul(pax[:, ph, :],
                                 xr[:, ph * 128:(ph + 1) * 128], sqT_ap,
                                 start=True, stop=True)
            axf = axfp.tile([128, KC, 128], dt_mm, tag="axf")
            nc.vector.tensor_copy(axf[:], pax[:, :, 0:128])
            hps = pH.tile([128, NF], F32, tag="hps")
            for off, wd in NSPLITS:
                for k in range(KC):
                    nc.tensor.matmul(hps[:, off:off + wd], axf[:, k, :],
                                     gwt[:, i, k, off:off + wd],
                                     start=(k == 0), stop=(k == KC - 1))
            # LayerNorm (+relu, +optional g/b) with inv_rs folded in
            hc = hcp.tile([128, NF], F32, tag="hc")
            s_sum = lnp.tile([128, 1], F32, tag="s_sum")
            nc.vector.tensor_scalar(hc[:], hps[:], invrs_ap, 0.0, ALU.mult,
                                    ALU.add, accum_out=s_sum[:])
            sqd = hcp.tile([128, NF], F32, tag="sqd")
            ssq = lnp.tile([128, 1], F32, tag="ssq")
            nc.scalar.activation(sqd[:], hc[:], AF.Square, accum_out=ssq[:])
            mu = lnp.tile([128, 1], F32, tag="mu")
            nc.vector.tensor_scalar_mul(mu[:], s_sum[:], 1.0 / NF)
            musq = lnp.tile([128, 1], F32, tag="musq")
            nc.vector.tensor_tensor(musq[:], mu[:], mu[:], ALU.mult)
            var = lnp.tile([128, 1], F32, tag="var")
            nc.vector.tensor_scalar(var[:], ssq[:], 1.0 / NF, musq[:],
                                    ALU.mult, ALU.subtract)
            std = lnp.tile([128, 1], F32, tag="std")
            nc.scalar.activation(std[:], var[:], AF.Sqrt, bias=epsb[:])
            rstd = lnp.tile([128, 1], F32, tag="rstd")
            nc.vector.reciprocal(rstd[:], std[:])
            nmr = lnp.tile([128, 1], F32, tag="nmr")
            nc.vector.tensor_scalar(nmr[:], mu[:], rstd[:], -1.0,
                                    ALU.mult, ALU.mult)
            xn = xnp.tile([128, NF], dt_ax, tag="xn")
            if ln_trivial:
                nc.scalar.activation(xn[:], hc[:], AF.Relu,
                                     bias=nmr[:], scale=rstd[:])
            else:
                t1 = hcp.tile([128, NF], F32, tag="t1")
                nc.scalar.activation(t1[:], hc[:], AF.Identity,
                                     bias=nmr[:], scale=rstd[:])
                nc.vector.tensor_tensor(t1[:], t1[:], lngt[:, i, :], ALU.mult)
                nc.vector.tensor_tensor(t1[:], t1[:], lnbt[:, i, :], ALU.add)
                nc.vector.tensor_scalar_max(xn[:], t1[:], 0.0)
            return xn

        with tc.tile_pool(name=P("x0rs"), bufs=3) as x0rp, \
             tc.tile_pool(name=P("axf"), bufs=3) as axfp, \
             tc.tile_pool(name=P("hc"), bufs=3) as hcp, \
             tc.tile_pool(name=P("xn"), bufs=3) as xnp, \
             tc.tile_pool(name=P("ln"), bufs=4) as lnp, \
             tc.tile_pool(name=P("pAX"), bufs=1, space="PSUM") as pAX, \
             tc.tile_pool(name=P("pH"), bufs=2, space="PSUM") as pH, \
             tc.tile_pool(name=P("pMean"), bufs=1, space="PSUM") as pM:
            x2f_ps = pM.tile([128, KC, G1], F32)  # [128,6,64] = 1536B: 1 bank
            for c in range(NCH):
                xr = x0rp.tile([128, NF], dt_ax, tag="x0rc")
                nc.sync.dma_start(xr[:], x0r_d[c])
                for i in range(L):
                    xr = gcn_layer(i, xr, c, sqT[:, c, :], invrs[:, c:c + 1],
                                   axfp, pAX, pH, hcp, lnp, xnp)
                for k in range(KC):
                    nc.tensor.matmul(x2f_ps[:, k, NV * c:NV * (c + 1)],
                                     axc(xr[:, k * 128:(k + 1) * 128]),
                                     mv[:], start=True, stop=True)

            # ================= PHASE C =================
            # PSUM reuses phase-B pools (pAX/pH/pM tags) to stay in 8 banks.
            with tc.tile_pool(name=P("s2"), bufs=1) as s2, \
                 tc.tile_pool(name=P("wstr"), bufs=4) as wstr:
                x2f = s2.tile([128, KC, G1], dt_mm)
                nc.vector.tensor_copy(x2f[:], x2f_ps[:])
                x2ft = s2.tile([128, KC, G1], F32)
                nc.vector.tensor_copy(x2ft[:], x2f_ps[:])
                # X2_R [128, NF] with g0 rows at 0:32, g1 at 64:96
                x2r = s2.tile([128, NF], dt_ax)
                if dt_ax != F32R:
                    nc.gpsimd.memset(x2r[:], 0.0)
                else:
                    nc.sync.dma_start(x2r[:], zeros_d[:, 0:NF])
                stg = s2.tile([64, KC, 128], dt_ax)
                for k in range(KC):
                    tp = pH.tile([128, 128], F32, tag="hps")
                    nc.tensor.transpose(tp[0:64, :], x2ft[:, k, :], ident[:])
                    nc.vector.tensor_copy(stg[:, k, :], tp[0:64, :])
                nc.vector.tensor_copy(x2r[0:32, :], stg[0:32])
                nc.sync.dma_start(x2r[64:96, :], stg[32:64])
                # TMP2 + dotT2 + sq2 + rs2
                tmpf2 = s2.tile([128, KC, G1], dt_mm)
                for lch in range(KC):
                    p2 = pAX.tile([128, G1], F32, tag="pax")
                    for k in range(KC):
                        nc.tensor.matmul(p2[:],
                                         mMt[:, k, lch * 128:(lch + 1) * 128],
                                         x2f[:, k, :],
                                         start=(k == 0), stop=(k == KC - 1))
                    nc.vector.tensor_copy(tmpf2[:, lch, :], p2[:])
                for g in range(NV):
                    po = 64 * g
                    sl = slice(32 * g, 32 * (g + 1))
                    pd2 = pAX.tile([128, 32], F32, tag="pax")
                    for k in range(KC):
                        nc.tensor.matmul(pd2[po:po + 32, :],
                                         mmc(x2f[:, k, sl]),
                                         mmc(tmpf2[:, k, sl]),
                                         start=(k == 0), stop=(k == KC - 1),
                                         tile_position=(0, po))
                    nc.scalar.activation(sqT2[po:po + 32, po:po + 32],
                                         pd2[po:po + 32, :], AF.Square)
                pr2 = pM.tile([128, 1], F32, tag="x2f_ps")
                nc.tensor.matmul(pr2[:], axc(sqT2[:, 0:128]), ones[:], start=True, stop=True)
                t2 = s2.tile([128, 1], F32)
                nc.vector.tensor_scalar_max(t2[:], pr2[:], L1_EPS)
                nc.vector.reciprocal(invrs2[:], t2[:])

                xr2 = x2r
                for i in range(L):
                    xr2 = gcn_layer(i, xr2, None, sqT2[:], invrs2[:],
                                    axfp, pAX, pH, hcp, lnp, xnp)
                x3_ps = pM.tile([128, KC, NV], F32, tag="x2f_ps")
                for k in range(KC):
                    nc.tensor.matmul(x3_ps[:, k, :],
                                     axc(xr2[:, k * 128:(k + 1) * 128]),
                                     mv2[:], start=True, stop=True)
                x3sb = s2.tile([128, KC, NV], F32)
                nc.vector.tensor_copy(x3sb[:], x3_ps[:])
                nc.sync.dma_start(x3_out.rearrange("k p v -> p k v"), x3sb[:])

                # -------- classifier --------
                fgt = s2.tile([128, KC, NV], F32)
                nc.sync.dma_start(fgt[:], fg_d.rearrange("k p v -> p k v"))
                f1bt = s2.tile([128, KC, 1], F32)
                nc.sync.dma_start(f1bt[:], f1b_d.rearrange("k p x -> p k x"))
                f2bt = s2.tile([128, 4, 1], F32)
                nc.sync.dma_start(f2bt[:], f2b_d.rearrange("m p x -> p m x"))
                f2r = s2.tile([128, KC, NC], F32)
                nc.sync.dma_start(f2r[:], f2t_d.rearrange("k p n -> p k n"))
                h1sb = s2.tile([128, KC, NV], F32)
                for mch in range(KC):
                    ph1 = pM.tile([128, NV], F32, tag="x2f_ps")
                    for j in range(2 * KC):
                        f1c = wstr.tile([128, 128], F32, tag="f1c")
                        nc.sync.dma_start(
                            f1c[:], f1t_d[j, :, mch * 128:(mch + 1) * 128])
                        src = x3sb[:, j, :] if j < KC else fgt[:, j - KC, :]
                        nc.tensor.matmul(ph1[:], f1c[:], src,
                                         start=(j == 0), stop=(j == 2 * KC - 1))
                    nc.scalar.activation(h1sb[:, mch, :], ph1[:],
                                         AF.Relu, bias=f1bt[:, mch, :])
                lgsb = s2.tile([128, 4, NV], F32)
                for mu in range(4):
                    wd = 128 if mu < 3 else NC - 384
                    pl = pM.tile([128, NV], F32, tag="x2f_ps")
                    for k in range(KC):
                        nc.tensor.matmul(pl[0:wd, :],
                                         f2r[:, k, mu * 128:mu * 128 + wd],
                                         h1sb[:, k, :],
                                         start=(k == 0), stop=(k == KC - 1))
                    nc.scalar.activation(lgsb[:, mu, :], pl[:],
                                         AF.Identity, bias=f2bt[:, mu, :])
                nc.sync.dma_start(lg_out.rearrange("m p v -> p m v"), lgsb[:])

    with tile.TileContext(nc) as tc:
        for R in range(repeats):
            with ExitStack() as ctx:
                emit(tc, ctx, R)
    nc.compile()
    return nc


# ====================== host side ======================

def _np(dt):
    return np.dtype(mybir.dt.np(dt))


def prep_core_inputs(feats, fg, wq_w, wk_w, gcn_w, ln_g, ln_b,
                     fc1_w, fc1_b, fc2_w, fc2_b,
                     ln_trivial=True, dt_mm=F32R, dt_ax=F32):
    """feats: [NV, FR, B, NF] (one core's shard), fg: [NV, NF].
    Returns dict name->np.ndarray for run_bass_kernel_spmd."""
    mm, ax = _np(dt_mm), _np(dt_ax)
    X0 = np.ascontiguousarray(feats.reshape(G1 * B, NF))
    x0f = np.ascontiguousarray(X0.T.reshape(KC, 128, G1 * B)).astype(mm)
    x0r = np.zeros((NCH, 128, NF), ax)
    xg = X0.reshape(G1, B, NF)
    for c in range(NCH):
        x0r[c, 0:B] = xg[2 * c]
        x0r[c, 64:64 + B] = xg[2 * c + 1]
    mM = (wq_w.astype(np.float64).T @ wk_w.astype(np.float64)).astype(np.float32)
    mM = np.ascontiguousarray(mM.reshape(KC, 128, NF)).astype(mm)
    gw = np.ascontiguousarray(gcn_w.reshape(L, KC, 128, NF)).astype(mm)
    f1t = np.ascontiguousarray(fc1_w.T.reshape(2 * KC, 128, NF)).astype(np.float32)
    f1b = np.ascontiguousarray(fc1_b.reshape(KC, 128, 1)).astype(np.float32)
    f2t = np.ascontiguousarray(fc2_w.T.reshape(KC, 128, NC)).astype(np.float32)
    f2b = np.zeros((4, 128, 1), np.float32)
    f2b.reshape(512)[:NC] = fc2_b
    fgc = np.ascontiguousarray(fg.T.reshape(KC, 128, NV)).astype(np.float32)
    sm = np.dtype(mybir.dt.np(BF16)) if dt_ax == BF16 else np.float32
    ones = np.ones((128, 1), sm)
    mv = np.zeros((128, NV), sm)
    for j in range(NV):
        mv[64 * j:64 * j + B, j] = np.float32(1.0 / B)
    mv2 = np.zeros((128, NV), sm)
    for j in range(NV):
        mv2[64 * j:64 * j + FR, j] = np.float32(1.0 / FR)
    zeros = np.zeros((128, NCH * SQW), ax)
    ident = np.eye(128, dtype=np.float32)
    d = dict(x0f=x0f, x0r=x0r, mM=mM, gw=gw, f1t=f1t, f1b=f1b, f2t=f2t,
             f2b=f2b, fg=fgc, ones=ones, mv=mv, mv2=mv2, ident=ident,
             zeros=zeros)
    if not ln_trivial:
        d["lng"] = np.broadcast_to(ln_g[:, None, :], (L, 128, NF)).astype(np.float32).copy()
        d["lnb"] = np.broadcast_to(ln_b[:, None, :], (L, 128, NF)).astype(np.float32).copy()
    return d


def assemble_outputs(results, feat_global):
    """results: per-core dicts with lgT [4,128,NV], x3T [6,128,NV]."""
    N = N_CORES * NV
    logits = np.zeros((N, NC), np.float32)
    x3 = np.zeros((N, NF), np.float32)
    for ci, r in enumerate(results):
        lg = r["lgT"].reshape(512, NV)
        x = r["x3T"].reshape(NF, NV)
        for v in range(NV):
            logits[ci * NV + v] = lg[:NC, v]
            x3[ci * NV + v] = x[:, v]
    y = np.concatenate([x3, feat_global.astype(np.float32)], axis=1)
    return logits, y


def run(inputs, nc=None, ln_trivial=True, dt_mm=F32R, dt_ax=F32):
    feats = np.asarray(inputs["feats"], np.float32)
    fg = np.asarray(inputs["feat_global_single"], np.float32)
    args = [np.asarray(inputs[k], np.float32) for k in
            ["wq_w", "wk_w", "gcn_w", "ln_g", "ln_b",
             "fc1_w", "fc1_b", "fc2_w", "fc2_b"]]
    if nc is None:
        nc = build(ln_trivial=ln_trivial, dt_mm=dt_mm, dt_ax=dt_ax)
    in_maps = []
    for ci in range(N_CORES):
        in_maps.append(prep_core_inputs(
            feats[ci * NV:(ci + 1) * NV], fg[ci * NV:(ci + 1) * NV], *args,
            ln_trivial=ln_trivial, dt_mm=dt_mm, dt_ax=dt_ax))
    res = run_bass_kernel_spmd(nc, in_maps, list(range(N_CORES)))
    return assemble_outputs(res.results, fg), nc, in_maps


# ====================== harness entrypoint ======================

_NC_CACHE = {}


def _get_nc(ln_trivial):
    key = ln_trivial
    if key not in _NC_CACHE:
        _NC_CACHE[key] = build(ln_trivial=ln_trivial)
    return _NC_CACHE[key]


def _numpy_reference(feats, feat_global_single, wq_w, wq_b, wk_w, wk_b,
                     gcn_w, ln_g, ln_b, fc1_w, fc1_b, fc2_w, fc2_b):
    """Pure-numpy fallback (only used for inputs outside the fast path's
    assumptions, i.e. nonzero attention biases)."""
    def graph(x):
        qx = x @ wq_w.T + wq_b
        kx = x @ wk_w.T + wk_b
        dot = np.einsum('gnd,gmd->gnm', qx, kx)
        sq = dot * dot
        adj = sq / np.maximum(sq.sum(-1, keepdims=True), L1_EPS)
        for i in range(gcn_w.shape[0]):
            h = adj @ (x @ gcn_w[i])
            mu = h.mean(-1, keepdims=True)
            var = ((h - mu) ** 2).mean(-1, keepdims=True)
            x = np.maximum((h - mu) / np.sqrt(var + LN_EPS) * ln_g[i] + ln_b[i], 0)
        return x.mean(-2)
    N, FRl, Bl, NFl = feats.shape
    x = graph(feats.reshape(N * FRl, Bl, NFl).astype(np.float64))
    x = graph(x.reshape(N, FRl, NFl))
    y = np.concatenate([x, feat_global_single.astype(np.float64)], axis=-1)
    h = np.maximum(y @ fc1_w.T + fc1_b, 0)
    logits = h @ fc2_w.T + fc2_b
    return logits.astype(np.float32), y.astype(np.float32)


def kernel(feats, feat_global_single, feat_single_previous=None,
           wq_w=None, wq_b=None, wk_w=None, wk_b=None, gcn_w=None,
           ln_g=None, ln_b=None, fc1_w=None, fc1_b=None,
           fc2_w=None, fc2_b=None, t_c=None, **_unused):
    feats = np.asarray(feats, np.float32)
    fg = np.asarray(feat_global_single, np.float32)
    wq_w = np.asarray(wq_w, np.float32); wk_w = np.asarray(wk_w, np.float32)
    wq_b = np.asarray(wq_b, np.float32); wk_b = np.asarray(wk_b, np.float32)
    gcn_w = np.asarray(gcn_w, np.float32)
    ln_g = np.asarray(ln_g, np.float32); ln_b = np.asarray(ln_b, np.float32)
    fc1_w = np.asarray(fc1_w, np.float32); fc1_b = np.asarray(fc1_b, np.float32)
    fc2_w = np.asarray(fc2_w, np.float32); fc2_b = np.asarray(fc2_b, np.float32)

    if np.any(wq_b != 0) or np.any(wk_b != 0):
        # attention biases break the precomputed wq^T.wk trick; rare path
        return _numpy_reference(feats, fg, wq_w, wq_b, wk_w, wk_b, gcn_w,
                                ln_g, ln_b, fc1_w, fc1_b, fc2_w, fc2_b)

    ln_trivial = bool(np.all(ln_g == 1.0) and np.all(ln_b == 0.0))
    nc = _get_nc(ln_trivial)
    in_maps = []
    for ci in range(N_CORES):
        in_maps.append(prep_core_inputs(
            feats[ci * NV:(ci + 1) * NV], fg[ci * NV:(ci + 1) * NV],
            wq_w, wk_w, gcn_w, ln_g, ln_b, fc1_w, fc1_b, fc2_w, fc2_b,
            ln_trivial=ln_trivial))
    res = run_bass_kernel_spmd(nc, in_maps, list(range(N_CORES)))
    logits, y = assemble_outputs(res.results, fg)
    return logits, y
